# revision 1
# baseline (speedup 1.0000x reference)
"""RGCN-style multi-relation GraphConv kernel for one TRN2 chip (8 NeuronCores).

Math (per relation r):  Z += D_in^{-1/2} A_r D_out^{-1/2} X W_r
Strategy:
  - Shard destination nodes across 8 cores (12500 rows each), graph-parallel.
  - Host: compute degrees + per-edge weight w_e = rsqrt(deg_out[src])*rsqrt(deg_in[dst]),
    bucket edges by (core, src-bank, dst-block of 128, relation), pad each segment to a
    multiple of 128 tokens (uniform across cores -> one SPMD program).
  - Device per core: bulk-gather X[src] rows (bf16) with gpsimd.dma_gather
    (int16 indices => 4 source banks of 32768 rows), build a weighted one-hot
    [edge, dst_local] tile on DVE (iota == dstloc) * w, aggregate with TensorE:
    PSUM[feat, dst] += Xg^T-contraction, i.e. matmul(lhsT=Xg_tile, rhs=onehot).
    Then Z^T[fout, dst] = sum_r W_r^T-contraction via matmul(lhsT=W_r, rhs=aggT_r).
  - Output Z^T per core -> host transposes/concats.
"""
import sys
sys.path.insert(0, "/opt/trn_rl_repo")
import numpy as np
import ml_dtypes

import concourse.bass as bass
import concourse.mybir as mybir
import concourse.tile as tile
from concourse import bacc
from concourse.bass_utils import run_bass_kernel_spmd

N_NODES = 100000
N_REL = 4
D = 128
NCORE = 8
NPC = N_NODES // NCORE          # 12500 dst rows per core
NB = (NPC + 127) // 128         # 98 dst blocks per core
BANK = 32768
NBANK = (N_NODES + BANK - 1) // BANK  # 4
CT = 32                         # 128-token tiles per gather chunk (4096 tokens)

BF16 = ml_dtypes.bfloat16

_cache: dict = {}


def _build(seglen128: np.ndarray, L_k: np.ndarray, GB=3, OB=3):
    """Build+compile the SPMD program. seglen128: [NBANK, NB, N_REL] tokens per
    segment (multiple of 128, uniform across cores). L_k: per-bank stream lengths."""
    nc = bacc.Bacc("TRN2", target_bir_lowering=False, debug=False, num_swdge_queues=4)
    xb = nc.dram_tensor("xb", [N_NODES, D], mybir.dt.bfloat16, kind="ExternalInput")
    idx16 = nc.dram_tensor("idx16", [128, int(L_k.sum()) // 16], mybir.dt.int16, kind="ExternalInput")
    dlv = nc.dram_tensor("dlv", [128, int(L_k.sum()) // 128], mybir.dt.bfloat16, kind="ExternalInput")
    wv = nc.dram_tensor("wv", [128, int(L_k.sum()) // 128], mybir.dt.bfloat16, kind="ExternalInput")
    iota = nc.dram_tensor("iota", [128, CT * 128], mybir.dt.bfloat16, kind="ExternalInput")
    wmat = nc.dram_tensor("wmat", [N_REL, D, D], mybir.dt.bfloat16, kind="ExternalInput")
    out = nc.dram_tensor("out", [128, NB * 128], mybir.dt.float32, kind="ExternalOutput")

    # per-bank column offsets into the concatenated streams
    bank_idx_off = np.concatenate([[0], np.cumsum(L_k // 16)])
    bank_tile_off = np.concatenate([[0], np.cumsum(L_k // 128)])
    ntiles_k = (L_k // 128).astype(int)
    nchunks_k = [(ntiles_k[k] + CT - 1) // CT for k in range(NBANK)]
    bank_rows = [min(BANK, N_NODES - k * BANK) for k in range(NBANK)]

    # segment -> (bank-local) tile ids
    flat = seglen128.reshape(NBANK, NB * N_REL)
    ends = flat.cumsum(axis=1)
    BO = (ends - flat)  # token start offsets per (k, b*4+r)

    with tile.TileContext(nc) as tc:
        import contextlib
        with contextlib.ExitStack() as ctx:
            const_p = ctx.enter_context(tc.tile_pool(name="const", bufs=1))
            g_pools = [ctx.enter_context(tc.tile_pool(name=f"g{k}", bufs=GB)) for k in range(NBANK)]
            i_pools = [ctx.enter_context(tc.tile_pool(name=f"i{k}", bufs=3)) for k in range(NBANK)]
            d_pools = [ctx.enter_context(tc.tile_pool(name=f"d{k}", bufs=3)) for k in range(NBANK)]
            w_pools = [ctx.enter_context(tc.tile_pool(name=f"w{k}", bufs=3)) for k in range(NBANK)]
            oh_pools = [ctx.enter_context(tc.tile_pool(name=f"oh{k}", bufs=OB)) for k in range(NBANK)]
            agg_ps = ctx.enter_context(tc.tile_pool(name="aggp", bufs=6, space="PSUM"))
            z_ps = ctx.enter_context(tc.tile_pool(name="zp", bufs=2, space="PSUM"))
            aggT_p = ctx.enter_context(tc.tile_pool(name="aggT", bufs=10))
            zo_p = ctx.enter_context(tc.tile_pool(name="zo", bufs=3))

            iota_sb = const_p.tile([128, CT, 128], mybir.dt.bfloat16, tag="iota")
            nc.sync.dma_start(iota_sb[:], iota[:])
            w_sb = const_p.tile([128, N_REL * 128], mybir.dt.bfloat16, tag="wmat")
            for r in range(N_REL):
                nc.sync.dma_start(w_sb[:, r * 128:(r + 1) * 128], wmat[r])

            chunks = [[None] * nchunks_k[k] for k in range(NBANK)]  # (g, dl, wv) tiles
            issued = [0] * NBANK

            def issue_chunk(k):
                ci = issued[k]
                ntok = min(CT * 128, ntiles_k[k] * 128 - ci * CT * 128)
                nt = ntok // 128
                it = i_pools[k].tile([128, CT * 8], mybir.dt.int16, tag=f"i{k}")
                c0 = bank_idx_off[k] + ci * CT * 8
                nc.sync.dma_start(it[:, :ntok // 16], idx16[:, c0:c0 + ntok // 16])
                t0 = bank_tile_off[k] + ci * CT
                dl = d_pools[k].tile([128, CT, 1], mybir.dt.bfloat16, tag=f"d{k}")
                nc.sync.dma_start(dl[:, :nt, 0], dlv[:, t0:t0 + nt])
                wt = w_pools[k].tile([128, CT, 1], mybir.dt.bfloat16, tag=f"w{k}")
                nc.sync.dma_start(wt[:, :nt, 0], wv[:, t0:t0 + nt])
                g = g_pools[k].tile([128, CT, D], mybir.dt.bfloat16, tag=f"g{k}")
                nc.gpsimd.dma_gather(
                    g[:, :nt, :], xb[k * BANK:k * BANK + bank_rows[k], :],
                    it[:, :ntok // 16], ntok, ntok, D, single_packet=False,
                    queue_num=k)
                oh = oh_pools[k].tile([128, CT, 128], mybir.dt.bfloat16, tag=f"oh{k}")
                nc.vector.tensor_tensor(
                    out=oh[:, :nt, :], in0=iota_sb[:, :nt, :],
                    in1=dl[:, :nt, :].to_broadcast([128, nt, 128]),
                    op=mybir.AluOpType.is_equal)
                nc.vector.tensor_tensor(
                    out=oh[:, :nt, :], in0=oh[:, :nt, :],
                    in1=wt[:, :nt, :].to_broadcast([128, nt, 128]),
                    op=mybir.AluOpType.mult)
                chunks[k][ci] = (g, oh)
                issued[k] = ci + 1

            for b in range(NB):
                aggs = []
                for r in range(N_REL):
                    # tiles of this (b, r) per bank
                    tiles = []
                    for k in range(NBANK):
                        s = int(BO[k, b * N_REL + r]) // 128
                        n = int(seglen128[k, b, r]) // 128
                        for j in range(n):
                            tiles.append((k, s + j))
                    # make sure chunks are issued
                    for (k, t) in tiles:
                        while issued[k] <= t // CT:
                            issue_chunk(k)
                    psum = agg_ps.tile([128, 128], mybir.dt.float32, tag="agg")
                    for i, (k, t) in enumerate(tiles):
                        g, oh = chunks[k][t // CT]
                        sl = t % CT
                        nc.tensor.matmul(psum[:], g[:, sl, :], oh[:, sl, :],
                                         start=(i == 0), stop=(i == len(tiles) - 1))
                    a = aggT_p.tile([128, 128], mybir.dt.bfloat16, tag="aggT")
                    if tiles:
                        nc.vector.tensor_copy(a[:], psum[:])
                    else:
                        nc.vector.memset(a[:], 0.0)
                    aggs.append(a)
                zp = z_ps.tile([128, 128], mybir.dt.float32, tag="z")
                for r in range(N_REL):
                    nc.tensor.matmul(zp[:], w_sb[:, r * 128:(r + 1) * 128], aggs[r][:],
                                     start=(r == 0), stop=(r == N_REL - 1))
                zo = zo_p.tile([128, 128], mybir.dt.float32, tag="zo")
                nc.vector.tensor_copy(zo[:], zp[:])
                nc.sync.dma_start(out[:, b * 128:(b + 1) * 128], zo[:])
    nc.compile()
    return nc


def _preprocess(edges, X, W):
    E = edges.shape[2]
    src = np.concatenate([edges[r, 0] for r in range(N_REL)]).astype(np.int64)
    dst = np.concatenate([edges[r, 1] for r in range(N_REL)]).astype(np.int64)
    rel = np.repeat(np.arange(N_REL), E)
    wlist = []
    for r in range(N_REL):
        dg_o = np.bincount(edges[r, 0], minlength=N_NODES).clip(1).astype(np.float64)
        dg_i = np.bincount(edges[r, 1], minlength=N_NODES).clip(1).astype(np.float64)
        wlist.append(1.0 / np.sqrt(dg_o[edges[r, 0]] * dg_i[edges[r, 1]]))
    w = np.concatenate(wlist).astype(np.float32)

    core = dst // NPC
    local = dst % NPC
    b = local // 128
    dloc = local % 128
    bank = src // BANK
    key = (((core * NBANK + bank) * NB + b) * N_REL + rel).astype(np.int64)
    order = np.argsort(key, kind="stable")
    key_s = key[order]
    NKEY = NCORE * NBANK * NB * N_REL
    cnt = np.bincount(key, minlength=NKEY)
    gstart = np.concatenate([[0], cnt.cumsum()])[:-1]
    ranks = np.arange(len(order)) - gstart[key_s]

    cnt4 = cnt.reshape(NCORE, NBANK, NB, N_REL)
    seglen128 = ((cnt4.max(axis=0) + 127) // 128) * 128  # [NBANK, NB, N_REL]
    flat = seglen128.reshape(NBANK, NB * N_REL)
    ends = flat.cumsum(axis=1)
    L_k = ends[:, -1].astype(np.int64)
    BO1 = (ends - flat).reshape(-1)  # indexed by (k, b*4+r)

    kk = key_s % (NBANK * NB * N_REL)
    pos = BO1[kk] + ranks  # position within (core, bank) stream
    src_s = src[order]
    dloc_s = dloc[order]
    w_s = w[order]
    core_s = core[order]
    bank_s = bank[order]

    idx16_maps, dl_maps, w_maps = [], [], []
    for c in range(NCORE):
        mcore = core_s == c
        idx_cols, dl_cols, w_cols = [], [], []
        for k in range(NBANK):
            m = mcore & (bank_s == k)
            Lk = int(L_k[k])
            a_idx = np.zeros(Lk, np.int16)
            a_dl = np.full(Lk, 255.0, np.float32)
            a_w = np.zeros(Lk, np.float32)
            p = pos[m]
            a_idx[p] = (src_s[m] - k * BANK).astype(np.int16)
            a_dl[p] = dloc_s[m]
            a_w[p] = w_s[m]
            idx_cols.append(np.tile(a_idx.reshape(-1, 16).T, (8, 1)))
            dl_cols.append(a_dl.reshape(-1, 128).T.astype(BF16))
            w_cols.append(a_w.reshape(-1, 128).T.astype(BF16))
        idx16_maps.append(np.ascontiguousarray(np.concatenate(idx_cols, axis=1)))
        dl_maps.append(np.ascontiguousarray(np.concatenate(dl_cols, axis=1)))
        w_maps.append(np.ascontiguousarray(np.concatenate(w_cols, axis=1)))

    return seglen128, L_k, idx16_maps, dl_maps, w_maps


def kernel(edges, X, W):
    edges = np.asarray(edges)
    X = np.asarray(X, dtype=np.float32)
    W = np.asarray(W, dtype=np.float32)

    seglen128, L_k, idx16_maps, dl_maps, w_maps = _preprocess(edges, X, W)

    ckey = seglen128.tobytes()
    if ckey not in _cache:
        try:
            _cache[ckey] = _build(seglen128, L_k, 3, 3)
        except ValueError:
            _cache[ckey] = _build(seglen128, L_k, 2, 2)
    nc = _cache[ckey]

    xb = np.ascontiguousarray(X.astype(BF16))
    iota_np = np.ascontiguousarray(
        np.broadcast_to(np.arange(128, dtype=np.float32), (128, CT, 128)).reshape(128, CT * 128)).astype(BF16)
    wmat = W.astype(BF16)
    in_maps = [
        {"xb": xb, "idx16": idx16_maps[c], "dlv": dl_maps[c], "wv": w_maps[c],
         "iota": iota_np, "wmat": wmat}
        for c in range(NCORE)
    ]
    res = run_bass_kernel_spmd(nc, in_maps, core_ids=list(range(NCORE)))
    Z = np.empty((N_NODES, D), np.float32)
    for c in range(NCORE):
        Z[c * NPC:(c + 1) * NPC] = np.asarray(res.results[c]["out"])[:, :NPC].T
    return Z

